# revision 1
# baseline (speedup 1.0000x reference)
"""GATEncoder kernel for 8 Trainium2 NeuronCores (bf16 edition).

Strategy (hardcoded for the nn_GATEncoder problem):
  - Only nodes < batch_size (8192) reach the output, so aggregation/decoder
    run for 8192 target nodes, sharded 1024 per core (8 windows of 128).
  - Encoder + GAT projection (xs, a_src, a_dst) are replicated on every core
    for all 10240 (padded) nodes; all matmuls run in bf16 (1 cyc/row on PE
    vs 4 for fp32) with fp32 PSUM accumulation.
  - Each core builds a node-major DRAM table T[10240, 640] bf16 =
    [xs_h0(256) | 1.0 | xs_h1(256) | 1.0 | a_src(2) | pad], in a per-core
    node permutation that puts the core's 1024 target nodes first. The
    constant-1 columns let the softmax denominator z ride along in the
    aggregation matmul (rhs slice is [xs_h | 1]).
  - The GAT projection runs node-major (lhsT = 128-node h2 chunk, rhs =
    gat weights), so no PE transposes are needed to build table rows.
  - Edges are partitioned by target core, bucketed into 8 windows of 128
    targets, sorted by source within a window, padded to a uniform
    per-window tile count (same static schedule on every core).
  - Per 128-edge tile: dma_gather pulls bf16 rows from T; one-hot matrices
    are built on-device from an iota/meta compare; attention softmax and
    scatter-add are one-hot matmuls into PSUM (exact for duplicate edges).
  - Epilogue (skip, ELU, decoder) feature-major on the local 1024 nodes.
"""

import math

import numpy as np
import ml_dtypes

N_NODES = 10000
NPAD = 10240
N_EDGES = 160000
N_IN, H, HEADS, HOUT = 128, 256, 2, 256
BATCH = 8192
NCORES = 8
TPC = BATCH // NCORES          # 1024 target nodes per core
P = 128
WPC = TPC // P                 # 8 windows per core
ROW = 640                      # bf16 table row (1280 B, %256 == 0)
F32 = np.float32
BF16 = ml_dtypes.bfloat16

_cache = {}


# ----------------------------------------------------------------------------
# Host-side preprocessing: edge partitioning / permutation / schedules
# ----------------------------------------------------------------------------

def _prepare_edges(edge_index):
    src = np.asarray(edge_index[0], dtype=np.int64)
    tgt = np.asarray(edge_index[1], dtype=np.int64)
    loops = np.arange(N_NODES, dtype=np.int64)
    src = np.concatenate([src, loops])
    tgt = np.concatenate([tgt, loops])
    keep = tgt < BATCH
    src, tgt = src[keep], tgt[keep]

    core = tgt // TPC
    tloc = tgt - core * TPC
    win = tloc // P
    trel = tloc - win * P

    # per (core, window) edge lists sorted by source
    buckets = {}
    counts = np.zeros((NCORES, WPC), dtype=np.int64)
    for c in range(NCORES):
        m = core == c
        sc, wc, rc = src[m], win[m], trel[m]
        for w in range(WPC):
            mw = wc == w
            s, r = sc[mw], rc[mw]
            o = np.argsort(s, kind="stable")
            buckets[(c, w)] = (s[o], r[o])
            counts[c, w] = s.size

    tiles_per_win = [int(math.ceil(counts[:, w].max() / P)) for w in range(WPC)]
    tiles_per_win = [max(t, 1) for t in tiles_per_win]
    return buckets, tiles_per_win


def _per_core_order(buckets, c):
    """Node permutation: the core's 1024 targets first, then the other
    nodes its edges actually source from (compaction)."""
    targets = np.arange(c * TPC, (c + 1) * TPC, dtype=np.int64)
    need = np.zeros(N_NODES, dtype=bool)
    for w in range(WPC):
        need[buckets[(c, w)][0]] = True
    need[targets] = False
    return np.concatenate([targets, np.nonzero(need)[0]])


def _per_core_arrays(buckets, tiles_per_win, c, order):
    """Returns (gather_idx int16 wrapped, tgt_rel f32 [P, TILES],
    ohT bf16 [TILES, j, p])."""
    ntiles = sum(tiles_per_win)
    srcs = np.zeros(ntiles * P, dtype=np.int64)      # padded slots gather row 0
    trel = np.full(ntiles * P, -1.0, dtype=F32)      # -1 -> contributes nothing
    t0 = 0
    for w in range(WPC):
        s, r = buckets[(c, w)]
        n = s.size
        base = t0 * P
        srcs[base : base + n] = s
        trel[base : base + n] = r.astype(F32)
        t0 += tiles_per_win[w]

    pos = np.zeros(N_NODES, dtype=np.int64)
    pos[order] = np.arange(order.size)

    gidx = pos[srcs].astype(np.int16)                # table row per edge slot
    # wrap int16 indices: element i at [i % 16, i // 16], replicated to 128 rows
    tot = gidx.size
    wrapped = gidx.reshape(tot // 16, 16).T          # [16, tot/16]
    wrapped = np.tile(wrapped, (8, 1)).copy()        # [128, tot/16]

    # per-tile max gather row (for row-limited gather dependencies)
    tmax = gidx.reshape(ntiles, P).max(axis=1).astype(np.int64)

    # one-hot libraries, bf16:
    #   ohJ[j, t*128+p] = (trel[t, p] == j)   (lhsT for dps: contract over j)
    #   ohP[p, t*128+j] = (trel[t, p] == j)   (row p scaled by wexp -> agg lhsT)
    tr = trel.reshape(ntiles, P)
    iota = np.arange(P, dtype=F32)
    ohT = (tr[:, None, :] == iota[None, :, None])       # [T, j, p]
    ohJ = np.ascontiguousarray(
        ohT.transpose(1, 0, 2).reshape(P, ntiles * P)).astype(BF16)
    ohP = np.ascontiguousarray(
        ohT.transpose(2, 0, 1).reshape(P, ntiles * P)).astype(BF16)
    return wrapped, ohJ, ohP, tmax


# ----------------------------------------------------------------------------
# Bass program
# ----------------------------------------------------------------------------

def _build_program(tiles_per_win, nb, trivial, rowlims):
    import concourse.bacc as bacc
    import concourse.mybir as mybir
    import concourse.tile as tile

    dt = mybir.dt
    Alu = mybir.AluOpType
    Act = mybir.ActivationFunctionType
    BF = dt.bfloat16

    TILES = sum(tiles_per_win)
    NB = nb                     # node blocks in phase A (compacted)
    BN = 512                    # nodes per block
    NT = BN // P                # 4 node chunks of 128 per block
    npad = NB * BN              # compacted node count

    nc = bacc.Bacc("TRN2", target_bir_lowering=False, num_swdge_queues=4)

    def inp(name, shape, dtype=dt.float32):
        return nc.dram_tensor(name, shape, dtype, kind="ExternalInput")

    xT = inp("xT", [P, npad], BF)
    w1T = inp("w1T", [N_IN, H], BF)
    ln_g = inp("ln_g", [H, 1])
    ln_b = inp("ln_b", [H, 1])
    b1 = inp("b1", [H, 1])
    b2 = inp("b2", [H, 1])
    w2T = inp("w2T", [H, H], BF)
    gatT = inp("gatT", [H, HEADS * HOUT + 4], BF)  # gat_w.T + att cols
    skipT = inp("skipT", [H, HEADS * HOUT], BF)
    gat_bias = inp("gat_bias", [HEADS * HOUT, 1])
    skip_b = inp("skip_b", [HEADS * HOUT, 1])
    d1T = inp("d1T", [HEADS * HOUT, 4 * H], BF)
    db1 = inp("db1", [4 * H, 1])
    d2T = inp("d2T", [4 * H, 1], BF)
    db2 = inp("db2", [1, 1])
    gidx = inp("gidx", [P, (TILES * P) // 16], dt.int16)
    ohT_in = inp("ohT", [P, TILES * P], BF)    # [j, t*128+p] one-hots
    ohP_in = inp("ohP", [P, TILES * P], BF)    # [p, t*128+j] one-hots
    negmean_in = inp("negmean", [P, P], BF)    # all -1/256
    posmean_in = inp("posmean", [P, P], BF)    # all +1/256
    ident_in = inp("ident", [P, P], BF)        # identity

    y_out = nc.dram_tensor("y", [1, TPC], dt.float32, kind="ExternalOutput")

    MM = HEADS * HOUT          # 512
    FC = MM // P               # 4 feature chunks of the GAT output
    HW = HOUT + 1              # 257: [xs_h | 1] rhs slice width

    with tile.TileContext(nc) as tc:
        with (
            tc.tile_pool(name="const", bufs=1) as cpool,
            tc.tile_pool(name="persist", bufs=1) as ppool,
            tc.tile_pool(name="dram", bufs=1, space="DRAM") as dpool,
        ):
            # ---- constants / weights to SBUF ----
            def load_const(t, shape, dtype=dt.float32):
                s = cpool.tile(shape, dtype, name=t.name, tag=t.name)
                nc.sync.dma_start(out=s[:], in_=t[:])
                return s

            def load_kc(t, k, cols, dtype=dt.float32):
                """[k*128, cols] DRAM -> [128, k, cols] SBUF."""
                s = cpool.tile([P, k, cols], dtype, name=t.name, tag=t.name)
                nc.sync.dma_start(
                    out=s[:], in_=t[:].rearrange("(k p) c -> p k c", p=P))
                return s

            negmean = load_const(negmean_in, [P, P], BF)
            posmean = load_const(posmean_in, [P, P], BF)
            w1s = load_const(w1T, [N_IN, H], BF)
            w2s = load_kc(w2T, 2, H, BF)
            gats = load_kc(gatT, 2, MM + 4, BF)
            lng = load_kc(ln_g, 2, 1)
            lnb = load_kc(ln_b, 2, 1)
            b1s = load_kc(b1, 2, 1)
            b2s = load_kc(b2, 2, 1)
            gidx_s = load_const(gidx, [P, (TILES * P) // 16], dt.int16)
            # one-hot library resident in SBUF: of-tile t = ohs[:, t*128:...]
            ohs = cpool.tile([P, TILES * P], BF, name="ohs", tag="ohs")
            ohp = cpool.tile([P, TILES * P], BF, name="ohp", tag="ohp")
            nohc = max(1, math.ceil(TILES / 16))
            for i in range(nohc):       # chunked so xb[0] isn't starved
                lo = i * (TILES * P) // nohc
                hi = (i + 1) * (TILES * P) // nohc
                nc.scalar.dma_start(out=ohs[:, lo:hi], in_=ohT_in[:, lo:hi])
                nc.scalar.dma_start(out=ohp[:, lo:hi], in_=ohP_in[:, lo:hi])
            ident = load_const(ident_in, [P, P], BF)
            skips = load_kc(skipT, 2, MM, BF)
            d1s = load_kc(d1T, 4, 4 * H, BF)
            d2s = load_kc(d2T, 8, 1, BF)
            gbia = load_kc(gat_bias, 4, 1)
            skb = load_kc(skip_b, 4, 1)
            db1s = load_kc(db1, 8, 1)
            db2s = load_const(db2, [1, 1])
            ln01 = cpool.tile([P, 1], dt.float32, name="ln01", tag="ln01")
            nc.gpsimd.memset(ln01[:], float(np.log(0.1)))

            T_tab = dpool.tile([npad, ROW], BF, name="T_tab", tag="T_tab")

            # persistent: local h2 (skip input), node-major a_dst, agg result
            h2loc = ppool.tile([P, 2, TPC], BF, name="h2loc", tag="h2loc")
            adstw = ppool.tile([P, 2 * WPC], BF, name="adstw", tag="adstw")
            aggs = ppool.tile([P, WPC, HEADS, HOUT], BF, name="aggs",
                              tag="aggs")

            # ================= Phase A: encoder -> table =================
            with (
                tc.tile_pool(name="wA", bufs=2) as wA,
                tc.tile_pool(name="asmp", bufs=2) as apool,
                tc.tile_pool(name="psA", bufs=1, space="PSUM") as psA,
                tc.tile_pool(name="psA1", bufs=1, space="PSUM") as psA1,
                tc.tile_pool(name="psA2", bufs=2, space="PSUM") as psA2,
                tc.tile_pool(name="psA3", bufs=1, space="PSUM") as psA3,
            ):
                for b in range(NB):
                    bsl = slice(b * BN, (b + 1) * BN)
                    xb = wA.tile([P, BN], BF, name="xb", tag="xb")
                    nc.sync.dma_start(out=xb[:], in_=xT[:, bsl])

                    ps2 = psA.tile([P, 2, BN], dt.float32, name="h1ps",
                                   tag="h1ps")
                    for m in range(2):
                        nc.tensor.matmul(
                            ps2[:, m, :], lhsT=w1s[:, m * P : (m + 1) * P],
                            rhs=xb[:], start=True, stop=True,
                            skip_group_check=True)
                    h1 = wA.tile([P, 2, BN], BF, name="h1", tag="h1")
                    if trivial:
                        nc.scalar.copy(h1[:], ps2[:])
                    else:
                        for m in range(2):
                            nc.scalar.activation(
                                h1[:, m, :], ps2[:, m, :], Act.Identity,
                                bias=b1s[:, m, 0:1])
                    sq = wA.tile([P, 2, BN], BF, name="sq", tag="sq")
                    nc.vector.scalar_tensor_tensor(
                        sq[:], h1[:], 1.0, h1[:],
                        op0=Alu.mult, op1=Alu.mult)

                    stats = psA1.tile([P, 2, BN], dt.float32, name="stats",
                                      tag="stats")
                    for m in range(2):
                        nc.tensor.matmul(stats[:, 0, :], lhsT=negmean[:],
                                         rhs=h1[:, m, :],
                                         start=(m == 0), stop=(m == 1),
                                         skip_group_check=True)
                        nc.tensor.matmul(stats[:, 1, :], lhsT=posmean[:],
                                         rhs=sq[:, m, :],
                                         start=(m == 0), stop=(m == 1),
                                         skip_group_check=True)
                    # stats[:,0]=-mean, stats[:,1]=E[h^2]
                    mu_bf = wA.tile([P, BN], BF, name="mu_bf", tag="mu_bf")
                    nc.vector.tensor_copy(mu_bf[:], stats[:, 0, :])
                    musq = wA.tile([P, BN], BF, name="musq", tag="musq")
                    nc.vector.scalar_tensor_tensor(
                        musq[:], mu_bf[:], 1.0, mu_bf[:],
                        op0=Alu.mult, op1=Alu.mult)
                    var = wA.tile([P, BN], dt.float32, name="var", tag="var")
                    nc.vector.scalar_tensor_tensor(
                        var[:], stats[:, 1, :], 1e-5, musq[:],
                        op0=Alu.add, op1=Alu.subtract)
                    rv = wA.tile([P, BN], dt.float32, name="rv", tag="rv")
                    nc.vector.reciprocal(rv[:], var[:])
                    rstd = wA.tile([P, BN], BF, name="rstd", tag="rstd")
                    nc.scalar.activation(rstd[:], rv[:], Act.Sqrt)

                    cen = wA.tile([P, 2, BN], BF, name="cen", tag="cen")
                    nc.vector.tensor_add(
                        cen[:], h1[:],
                        mu_bf[:].unsqueeze(1).to_broadcast([P, 2, BN]))
                    cn = wA.tile([P, 2, BN], BF, name="cn", tag="cn")
                    nc.vector.tensor_mul(
                        cn[:], cen[:],
                        rstd[:].unsqueeze(1).to_broadcast([P, 2, BN]))
                    hrelu = wA.tile([P, 2, BN], BF, name="hrelu", tag="hrelu")
                    if trivial:
                        nc.vector.tensor_scalar_max(hrelu[:], cn[:], 0.0)
                    else:
                        for m in range(2):
                            nc.scalar.activation(
                                hrelu[:, m, :], cn[:, m, :], Act.Relu,
                                bias=lnb[:, m, 0:1], scale=lng[:, m, 0:1])

                    ps2b = psA3.tile([P, 2, BN], dt.float32, name="h2ps",
                                     tag="h2ps")
                    for m in range(2):
                        for k in range(2):
                            nc.tensor.matmul(
                                ps2b[:, m, :],
                                lhsT=w2s[:, k, m * P : (m + 1) * P],
                                rhs=hrelu[:, k, :],
                                start=(k == 0), stop=(k == 1),
                                skip_group_check=True)
                    h2 = wA.tile([P, 2, BN], BF, name="h2", tag="h2")
                    if trivial:
                        nc.scalar.copy(h2[:], ps2b[:])
                    else:
                        for m in range(2):
                            nc.scalar.activation(
                                h2[:, m, :], ps2b[:, m, :], Act.Identity,
                                bias=b2s[:, m, 0:1])

                    if b * BN < TPC:  # blocks covering the local 1024 targets
                        lo = b * BN
                        nc.vector.tensor_copy(
                            h2loc[:, :, lo : lo + BN], h2[:])

                    # GAT projection, node-major: one 128-node chunk at a time
                    asm = apool.tile([P, NT, ROW], BF, name="asm", tag="asm")
                    nc.gpsimd.memset(asm[:, :, HOUT : HOUT + 1], 1.0)
                    nc.gpsimd.memset(asm[:, :, MM + 1 : MM + 2], 1.0)
                    avall = ps2b[:, 0, 0 : NT * 4].rearrange(
                        "p (t f) -> p t f", f=4)
                    for t in range(NT):
                        tsl = slice(t * P, (t + 1) * P)
                        xsps = psA2.tile([P, MM], dt.float32, name="xsps",
                                         tag="xsps")
                        for k in range(2):
                            nc.tensor.matmul(
                                xsps[:], lhsT=h2[:, k, tsl],
                                rhs=gats[:, k, 0:MM],
                                start=(k == 0), stop=(k == 1))
                        for k in range(2):
                            nc.tensor.matmul(
                                avall[:, t, :], lhsT=h2[:, k, tsl],
                                rhs=gats[:, k, MM : MM + 4],
                                start=(k == 0), stop=(k == 1),
                                skip_group_check=True)
                        # table row: [xs_h0 | 1 | xs_h1 | 1 | a_src | pad]
                        nc.scalar.copy(asm[:, t, 0:HOUT], xsps[:, 0:HOUT])
                        nc.vector.tensor_copy(asm[:, t, HOUT + 1 : MM + 1],
                                              xsps[:, HOUT:MM])
                    nc.vector.tensor_copy(asm[:, :, MM + 2 : MM + 4],
                                          avall[:, :, 0:2])
                    if b * BN < TPC:
                        nc.vector.tensor_copy(
                            adstw[:, b * 2 * NT : (b + 1) * 2 * NT]
                            .rearrange("p (t two) -> p t two", two=2),
                            avall[:, :, 2:4])
                    dst = T_tab[:].rearrange("(bb tt pp) r -> bb pp tt r",
                                             bb=NB, pp=P)[b]
                    nc.sync.dma_start(out=dst, in_=asm[:])

            # ================= Phase B: edge aggregation =================
            # Per window: half-window gathers interleaved with their
            # consumers; agg (with fused z column) accumulates in PSUM
            # across the whole window. Gathers carry row-limited in_aps so
            # early tiles can start while phase A is still writing T.
            win_t0 = []
            t0 = 0
            for w in range(WPC):
                win_t0.append(t0)
                t0 += tiles_per_win[w]
            GH = 8      # max tiles per gather call (1024 idx = 64 desc/engine)

            def _chunks(base, n):
                k = math.ceil(n / GH)
                sizes = [n // k + (1 if i < n % k else 0) for i in range(k)]
                out, b0 = [], base
                for s in sizes:
                    out.append((b0, s))
                    b0 += s
                return out

            with (
                tc.tile_pool(name="wB", bufs=3) as wB,
                tc.tile_pool(name="gpool", bufs=6) as gpool,
                tc.tile_pool(name="psB", bufs=2, space="PSUM") as psB,
            ):
                gcall = 0
                for w in range(WPC):
                    ntw = tiles_per_win[w]
                    halves = _chunks(win_t0[w], ntw)
                    agg = [psB.tile([P, HW], dt.float32, name=f"aggps{h}",
                                    tag=f"aggps{h}") for h in range(HEADS)]
                    done = 0
                    for hb, hn in halves:
                        if hn == 0:
                            continue
                        gb = gpool.tile([P, GH, ROW], BF, name="gb", tag="gb")
                        lim = rowlims[gcall] if rowlims else npad
                        nc.gpsimd.dma_gather(
                            out_ap=gb[:, :hn, :],
                            in_ap=T_tab[0:lim],
                            idxs_ap=gidx_s[:, hb * 8 : (hb + hn) * 8],
                            num_idxs=hn * P,
                            num_idxs_reg=hn * P,
                            elem_size=ROW,
                            queue_num=gcall % 4,
                        )
                        gcall += 1

                        dps = psB.tile([P, 2 * GH], dt.float32, name="dps",
                                       tag="dps")
                        for i in range(hn):
                            t = hb + i
                            nc.tensor.matmul(
                                dps[:, 2 * i : 2 * i + 2],
                                lhsT=ohs[:, t * P : (t + 1) * P],
                                rhs=adstw[:, 2 * w : 2 * w + 2],
                                start=(i == 0), stop=(i == hn - 1),
                                skip_group_check=True)
                        # e = a_src[src] + d ; leaky(0.2); exp
                        esb = wB.tile([P, GH, 2], dt.float32, name="esb",
                                      tag="esb")
                        nc.vector.tensor_add(
                            esb[:, :hn, :],
                            gb[:, :hn, MM + 2 : MM + 4],
                            dps[:, : 2 * hn].rearrange(
                                "p (t two) -> p t two", two=2))
                        lk = wB.tile([P, GH, 2], dt.float32, name="lk",
                                     tag="lk")
                        nc.vector.scalar_tensor_tensor(
                            lk[:, :hn, :], esb[:, :hn, :], 0.2,
                            esb[:, :hn, :], op0=Alu.mult, op1=Alu.max)
                        wexp = wB.tile([P, GH, 2], BF, name="wexp",
                                       tag="wexp")
                        nc.scalar.activation(wexp[:, :hn, :],
                                             lk[:, :hn, :], Act.Exp)

                        # weighted one-hots, batched per head on DVE
                        ohw = wB.tile([P, HEADS, GH, P], BF, name="ohw",
                                      tag="ohw")
                        ofv = ohp[:, hb * P : (hb + hn) * P].rearrange(
                            "p (t j) -> p t j", j=P)
                        for h in range(HEADS):
                            nc.vector.tensor_mul(
                                ohw[:, h, :hn, :], ofv,
                                wexp[:, :hn, h : h + 1]
                                .to_broadcast([P, hn, P]))

                        for i in range(hn):
                            for h in range(HEADS):
                                nc.tensor.matmul(
                                    agg[h][:],
                                    lhsT=ohw[:, h, i, :],
                                    rhs=gb[:, i, h * HW : (h + 1) * HW],
                                    start=(done == 0),
                                    stop=(done == ntw - 1),
                                    skip_group_check=True)
                            done += 1
                    # normalize: alpha = w / z  (z rode along in col 256)
                    rz = wB.tile([P, HEADS], dt.float32, name="rz", tag="rz")
                    for h in range(HEADS):
                        nc.vector.reciprocal(rz[:, h : h + 1],
                                             agg[h][:, HOUT : HOUT + 1])
                        nc.vector.tensor_scalar(
                            aggs[:, w, h, :], agg[h][:, 0:HOUT],
                            rz[:, h : h + 1], None, op0=Alu.mult)

            # ================= Phase C: epilogue =================
            with (
                tc.tile_pool(name="wC", bufs=1) as wC,
                tc.tile_pool(name="wC2", bufs=2) as wC2,
                tc.tile_pool(name="psC", bufs=2, space="PSUM") as psC,
                tc.tile_pool(name="psC2", bufs=1, space="PSUM") as psC2,
                tc.tile_pool(name="psCt", bufs=1, space="PSUM") as psCt,
            ):
                # aggs node-major [tgt, head, feat] -> convT feature-major
                convT = wC.tile([P, FC, TPC], BF, name="convT", tag="convT")
                for w in range(WPC):
                    tp = psCt.tile([P, FC, P], BF, name="tpC", tag="tpC")
                    for f in range(FC):
                        h, fo = divmod(f * P, HOUT)
                        nc.tensor.transpose(
                            tp[:, f, :], aggs[:, w, h, fo : fo + P],
                            ident[:])
                    if trivial:
                        nc.vector.tensor_copy(
                            convT[:, :, w * P : (w + 1) * P], tp[:])
                    else:
                        for f in range(FC):
                            nc.scalar.activation(
                                convT[:, f, w * P : (w + 1) * P],
                                tp[:, f, :], Act.Identity,
                                bias=gbia[:, f, 0:1])

                outT = wC.tile([P, FC, TPC], BF, name="outT", tag="outT")
                for f in range(FC):
                    sp = psC2.tile([P, 2, 512], dt.float32, name="skps",
                                   tag="skps")
                    for n in range(2):
                        for k in range(2):
                            nc.tensor.matmul(
                                sp[:, n, :],
                                lhsT=skips[:, k, f * P : (f + 1) * P],
                                rhs=h2loc[:, k, n * 512 : (n + 1) * 512],
                                start=(k == 0), stop=(k == 1),
                                skip_group_check=True)
                    cvv = convT[:, f, :].rearrange("p (n c) -> p n c", n=2)
                    t_sb = wC2.tile([P, 2, 512], BF, name="t_sb",
                                    tag="t_sb")
                    if trivial:
                        nc.vector.tensor_add(t_sb[:], sp[:], cvv)
                    else:
                        nc.vector.scalar_tensor_tensor(
                            t_sb[:], sp[:], skb[:, f, 0:1], cvv,
                            op0=Alu.add, op1=Alu.add)
                    mn = wC2.tile([P, 2, 512], BF, name="mn", tag="mn")
                    nc.vector.tensor_scalar_min(mn[:], t_sb[:], 0.0)
                    ez = wC2.tile([P, 2, 512], dt.float32, name="ez",
                                  tag="ez")
                    nc.scalar.activation(ez[:], mn[:], Act.Exp,
                                         bias=ln01[:, 0:1])
                    rl = wC2.tile([P, 2, 512], dt.float32, name="rl",
                                  tag="rl")
                    nc.scalar.activation(rl[:], t_sb[:], Act.Relu)
                    nc.vector.scalar_tensor_tensor(
                        outT[:, f, :].rearrange("p (n c) -> p n c", n=2),
                        ez[:], -0.1, rl[:], op0=Alu.add, op1=Alu.add)

                dsb = wC.tile([P, 8, TPC], BF, name="dsb", tag="dsb")
                for m in range(8):
                    ps = psC.tile([P, 2, 512], dt.float32, name="decps",
                                  tag="decps")
                    for n in range(2):
                        for k in range(FC):
                            nc.tensor.matmul(
                                ps[:, n, :],
                                lhsT=d1s[:, k, m * P : (m + 1) * P],
                                rhs=outT[:, k, n * 512 : (n + 1) * 512],
                                start=(k == 0), stop=(k == FC - 1),
                                skip_group_check=True)
                    tmp = wC2.tile([P, 2, 512], BF, name="dtmp",
                                   tag="dtmp")
                    if trivial:
                        nc.scalar.copy(tmp[:], ps[:])
                    else:
                        nc.scalar.activation(tmp[:], ps[:], Act.Identity,
                                             bias=db1s[:, m, 0:1])
                    nc.vector.scalar_tensor_tensor(
                        dsb[:, m, :].rearrange("p (n c) -> p n c", n=2),
                        tmp[:], 0.1, tmp[:], op0=Alu.mult, op1=Alu.max)

                ysb = wC.tile([1, TPC], dt.float32, name="ysb", tag="ysb")
                for n in range(2):
                    nsl = slice(n * 512, (n + 1) * 512)
                    yp = psC.tile([1, 512], dt.float32, name="yps",
                                  tag="yps", bufs=1)
                    for m in range(8):
                        nc.tensor.matmul(
                            yp[:], lhsT=d2s[:, m, 0:1],
                            rhs=dsb[:, m, nsl],
                            start=(m == 0), stop=(m == 7))
                    nc.scalar.activation(ysb[:, nsl], yp[:], Act.Identity,
                                         bias=db2s[0:1, 0:1])
                nc.sync.dma_start(out=y_out[:], in_=ysb[:])

    nc.compile()
    return nc


# ----------------------------------------------------------------------------
# Driver
# ----------------------------------------------------------------------------

def _consts():
    negmean = np.full((P, P), -1.0 / H, dtype=BF16)
    posmean = np.full((P, P), 1.0 / H, dtype=BF16)
    ident = np.eye(P, dtype=BF16)
    return negmean, posmean, ident


def _host_in_maps(inputs, buckets, tiles_per_win):
    x = np.asarray(inputs["x"], dtype=F32)
    enc_w1, enc_b1 = inputs["enc_w1"], inputs["enc_b1"]
    ln_g, ln_b = inputs["ln_g"], inputs["ln_b"]
    enc_w2, enc_b2 = inputs["enc_w2"], inputs["enc_b2"]
    gat_w, att_src, att_dst = inputs["gat_w"], inputs["att_src"], inputs["att_dst"]
    gat_bias, skip_w, skip_b = inputs["gat_bias"], inputs["skip_w"], inputs["skip_b"]
    dec_w1, dec_b1 = inputs["dec_w1"], inputs["dec_b1"]
    dec_w2, dec_b2 = inputs["dec_w2"], inputs["dec_b2"]

    trivial = bool(
        all(np.all(np.asarray(v, F32) == 0.0)
            for v in (enc_b1, enc_b2, ln_b, gat_bias, skip_b, dec_b1,
                      dec_b2))
        and np.all(np.asarray(ln_g, F32) == 1.0))

    # a_src[n,h] = att_src[h] . xs[n,h,:] = (gat_w[h-block].T @ att_src[h]) . h2
    # -> compose the attention vectors into h2-space columns on the host.
    gw = np.asarray(gat_w, F32)
    asr = np.asarray(att_src, F32).reshape(HEADS, HOUT)
    ads = np.asarray(att_dst, F32).reshape(HEADS, HOUT)
    att = np.zeros((H, 4), dtype=F32)
    for h in range(HEADS):
        blk = gw[h * HOUT : (h + 1) * HOUT, :]        # [HOUT, H]
        att[:, h] = blk.T @ asr[h]
        att[:, 2 + h] = blk.T @ ads[h]
    gatT = np.concatenate([np.ascontiguousarray(gw.T), att], axis=1)

    negmean, posmean, ident = _consts()
    col = lambda v: np.ascontiguousarray(np.asarray(v, F32).reshape(-1, 1))
    bf = lambda v: np.ascontiguousarray(np.asarray(v, F32)).astype(BF16)
    common = {
        "w1T": bf(np.asarray(enc_w1, F32).T),
        "ln_g": col(ln_g), "ln_b": col(ln_b),
        "b1": col(enc_b1), "b2": col(enc_b2),
        "w2T": bf(np.asarray(enc_w2, F32).T),
        "gatT": bf(gatT),
        "skipT": bf(np.asarray(skip_w, F32).T),
        "gat_bias": col(gat_bias), "skip_b": col(skip_b),
        "d1T": bf(np.asarray(dec_w1, F32).T),
        "db1": col(dec_b1),
        "d2T": bf(np.asarray(dec_w2, F32).T),
        "db2": col(dec_b2),
        "negmean": negmean, "posmean": posmean,
        "ident": ident,
    }

    orders = [_per_core_order(buckets, c) for c in range(NCORES)]
    nb = int(math.ceil(max(o.size for o in orders) / 512))
    npad = nb * 512

    in_maps = []
    tmaxes = []
    for c in range(NCORES):
        order = orders[c]
        wrapped, ohJ, ohP, tmax = _per_core_arrays(
            buckets, tiles_per_win, c, order)
        tmaxes.append(tmax)
        m = dict(common)
        xp = np.zeros((npad, N_IN), dtype=F32)
        xp[:order.size] = x[order]
        m["xT"] = np.ascontiguousarray(xp.T).astype(BF16)
        m["gidx"] = wrapped
        m["ohT"] = ohJ
        m["ohP"] = ohP
        in_maps.append(m)

    # per-gather-call row limits (same call order as the builder emits)
    tmax_all = np.max(np.stack(tmaxes), axis=0)      # per tile, over cores
    GH = 8
    rowlims = []
    t0 = 0
    for w in range(WPC):
        ntw = tiles_per_win[w]
        k = math.ceil(ntw / GH)
        sizes = [ntw // k + (1 if i < ntw % k else 0) for i in range(k)]
        b0 = t0
        for sz in sizes:
            hi = int(tmax_all[b0 : b0 + sz].max()) + 1
            rowlims.append(min(int(math.ceil(hi / 512.0)) * 512, npad))
            b0 += sz
        t0 += ntw
    return in_maps, nb, trivial, rowlims


def kernel(x, edge_index, batch_size, enc_w1, enc_b1, ln_g, ln_b, enc_w2,
           enc_b2, gat_w, att_src, att_dst, gat_bias, skip_w, skip_b,
           dec_w1, dec_b1, dec_w2, dec_b2, _trace=False):
    edge_index = np.asarray(edge_index)
    buckets, tiles_per_win = _prepare_edges(edge_index)

    inputs = dict(x=x, enc_w1=enc_w1, enc_b1=enc_b1, ln_g=ln_g, ln_b=ln_b,
                  enc_w2=enc_w2, enc_b2=enc_b2, gat_w=gat_w, att_src=att_src,
                  att_dst=att_dst, gat_bias=gat_bias, skip_w=skip_w,
                  skip_b=skip_b, dec_w1=dec_w1, dec_b1=dec_b1, dec_w2=dec_w2,
                  dec_b2=dec_b2)
    in_maps, nb, trivial, rowlims = _host_in_maps(
        inputs, buckets, tiles_per_win)

    key = (tuple(tiles_per_win), nb, trivial, tuple(rowlims))
    if key not in _cache:
        _cache[key] = _build_program(tiles_per_win, nb, trivial, rowlims)
    nc = _cache[key]

    from concourse.bass_utils import run_bass_kernel_spmd
    res = run_bass_kernel_spmd(
        nc, in_maps, core_ids=list(range(NCORES)), trace=_trace)

    y = np.concatenate([res.results[c]["y"][0] for c in range(NCORES)])
    out = y.reshape(BATCH, 1).astype(F32)
    if _trace:
        return out, res
    return out



# revision 27
# speedup vs baseline: 4.3339x; 4.3339x over previous
"""GATEncoder kernel for 8 Trainium2 NeuronCores (bf16 edition).

Strategy (hardcoded for the nn_GATEncoder problem):
  - Only nodes < batch_size (8192) reach the output, so aggregation/decoder
    run for 8192 target nodes, sharded 1024 per core (8 windows of 128).
  - Encoder + GAT projection (xs, a_src, a_dst) are replicated on every core
    for all 10240 (padded) nodes; all matmuls run in bf16 (1 cyc/row on PE
    vs 4 for fp32) with fp32 PSUM accumulation.
  - Each core builds a node-major DRAM table T[10240, 640] bf16 =
    [xs_h0(256) | 1.0 | xs_h1(256) | 1.0 | a_src(2) | pad], in a per-core
    node permutation that puts the core's 1024 target nodes first. The
    constant-1 columns let the softmax denominator z ride along in the
    aggregation matmul (rhs slice is [xs_h | 1]).
  - The GAT projection runs node-major (lhsT = 128-node h2 chunk, rhs =
    gat weights), so no PE transposes are needed to build table rows.
  - Edges are partitioned by target core, bucketed into 8 windows of 128
    targets, sorted by source within a window, padded to a uniform
    per-window tile count (same static schedule on every core).
  - Per 128-edge tile: dma_gather pulls bf16 rows from T; one-hot matrices
    are built on-device from an iota/meta compare; attention softmax and
    scatter-add are one-hot matmuls into PSUM (exact for duplicate edges).
  - Epilogue (skip, ELU, decoder) feature-major on the local 1024 nodes.
"""

import math

import numpy as np
import ml_dtypes

N_NODES = 10000
NPAD = 10240
N_EDGES = 160000
N_IN, H, HEADS, HOUT = 128, 256, 2, 256
BATCH = 8192
NCORES = 8
TPC = BATCH // NCORES          # 1024 target nodes per core
P = 128
WPC = TPC // P                 # 8 windows per core
ROW = 640                      # legacy bf16 table row (1280 B, %256 == 0)
ROWF = 384                     # fast-path row (768 B): [hrelu 256|1|a_src 2|pad]
F32 = np.float32
BF16 = ml_dtypes.bfloat16

_cache = {}


# ----------------------------------------------------------------------------
# Host-side preprocessing: edge partitioning / permutation / schedules
# ----------------------------------------------------------------------------

def _prepare_edges(edge_index):
    src = np.asarray(edge_index[0], dtype=np.int64)
    tgt = np.asarray(edge_index[1], dtype=np.int64)
    loops = np.arange(N_NODES, dtype=np.int64)
    src = np.concatenate([src, loops])
    tgt = np.concatenate([tgt, loops])
    keep = tgt < BATCH
    src, tgt = src[keep], tgt[keep]

    core = tgt // TPC
    tloc = tgt - core * TPC
    win = tloc // P
    trel = tloc - win * P

    # per (core, window) edge lists sorted by source
    buckets = {}
    counts = np.zeros((NCORES, WPC), dtype=np.int64)
    for c in range(NCORES):
        m = core == c
        sc, wc, rc = src[m], win[m], trel[m]
        for w in range(WPC):
            mw = wc == w
            s, r = sc[mw], rc[mw]
            o = np.argsort(s, kind="stable")
            buckets[(c, w)] = (s[o], r[o])
            counts[c, w] = s.size

    tiles_per_win = [int(math.ceil(counts[:, w].max() / P)) for w in range(WPC)]
    tiles_per_win = [max(t, 1) for t in tiles_per_win]
    return buckets, tiles_per_win


def _per_core_order(buckets, c):
    """Node permutation: the core's 1024 targets first, then the other
    nodes its edges actually source from (compaction)."""
    targets = np.arange(c * TPC, (c + 1) * TPC, dtype=np.int64)
    need = np.zeros(N_NODES, dtype=bool)
    for w in range(WPC):
        need[buckets[(c, w)][0]] = True
    need[targets] = False
    return np.concatenate([targets, np.nonzero(need)[0]])


def _per_core_arrays(buckets, tiles_per_win, c, order, pmajor):
    """Returns (gather_idx int16 wrapped, tgt_rel f32 [P, TILES],
    ohT bf16 [TILES, j, p]).

    pmajor: fast-path table row layout — order position i = b*512+t*128+p
    is stored at DRAM row b*512 + p*4 + t (one contiguous 4-row run per
    partition per block, so the table write is 128 descriptors/block)."""
    ntiles = sum(tiles_per_win)
    srcs = np.zeros(ntiles * P, dtype=np.int64)      # padded slots gather row 0
    trel = np.full(ntiles * P, -1.0, dtype=F32)      # -1 -> contributes nothing
    t0 = 0
    for w in range(WPC):
        s, r = buckets[(c, w)]
        n = s.size
        base = t0 * P
        srcs[base : base + n] = s
        trel[base : base + n] = r.astype(F32)
        t0 += tiles_per_win[w]

    pos = np.zeros(N_NODES, dtype=np.int64)
    i = np.arange(order.size)
    if pmajor:
        b, r = i // 512, i % 512
        rows = b * 512 + (r % P) * 4 + (r // P)
    else:
        rows = i
    pos[order] = rows

    gidx = pos[srcs].astype(np.int16)                # table row per edge slot
    # wrap int16 indices: element i at [i % 16, i // 16], replicated to 128 rows
    tot = gidx.size
    wrapped = gidx.reshape(tot // 16, 16).T          # [16, tot/16]
    wrapped = np.tile(wrapped, (8, 1)).copy()        # [128, tot/16]

    # per-tile max gather row (for row-limited gather dependencies)
    tmax = gidx.reshape(ntiles, P).max(axis=1).astype(np.int64)

    # one-hot libraries, bf16:
    #   ohJ[j, t*128+p] = (trel[t, p] == j)   (lhsT for dps: contract over j)
    #   ohP[p, t*128+j] = (trel[t, p] == j)   (row p scaled by wexp -> agg lhsT)
    tr = trel.reshape(ntiles, P)
    iota = np.arange(P, dtype=F32)
    ohT = (tr[:, None, :] == iota[None, :, None])       # [T, j, p]
    ohJ = np.ascontiguousarray(
        ohT.transpose(1, 0, 2).reshape(P, ntiles * P)).astype(BF16)
    ohP = np.ascontiguousarray(
        ohT.transpose(2, 0, 1).reshape(P, ntiles * P)).astype(BF16)
    return wrapped, ohJ, ohP, tmax


# ----------------------------------------------------------------------------
# Bass program
# ----------------------------------------------------------------------------

def _build_program_legacy(tiles_per_win, nb, trivial, rowlims):
    import concourse.bacc as bacc
    import concourse.mybir as mybir
    import concourse.tile as tile

    dt = mybir.dt
    Alu = mybir.AluOpType
    Act = mybir.ActivationFunctionType
    BF = dt.bfloat16

    TILES = sum(tiles_per_win)
    NB = nb                     # node blocks in phase A (compacted)
    BN = 512                    # nodes per block
    NT = BN // P                # 4 node chunks of 128 per block
    npad = NB * BN              # compacted node count

    nc = bacc.Bacc("TRN2", target_bir_lowering=False, num_swdge_queues=4)

    def inp(name, shape, dtype=dt.float32):
        return nc.dram_tensor(name, shape, dtype, kind="ExternalInput")

    xT = inp("xT", [P, npad], BF)
    w1T = inp("w1T", [N_IN, H], BF)
    ln_g = inp("ln_g", [H, 1])
    ln_b = inp("ln_b", [H, 1])
    b1 = inp("b1", [H, 1])
    b2 = inp("b2", [H, 1])
    w2T = inp("w2T", [H, H], BF)
    gatT = inp("gatT", [H, HEADS * HOUT + 4], BF)  # gat_w.T + att cols
    skipT = inp("skipT", [H, HEADS * HOUT], BF)
    gat_bias = inp("gat_bias", [HEADS * HOUT, 1])
    skip_b = inp("skip_b", [HEADS * HOUT, 1])
    d1T = inp("d1T", [HEADS * HOUT, 4 * H], BF)
    db1 = inp("db1", [4 * H, 1])
    d2T = inp("d2T", [4 * H, 1], BF)
    db2 = inp("db2", [1, 1])
    gidx = inp("gidx", [P, (TILES * P) // 16], dt.int16)
    ohT_in = inp("ohT", [P, TILES * P], BF)    # [j, t*128+p] one-hots
    ohP_in = inp("ohP", [P, TILES * P], BF)    # [p, t*128+j] one-hots
    negmean_in = inp("negmean", [P, P], BF)    # all -1/256
    posmean_in = inp("posmean", [P, P], BF)    # all +1/256
    ident_in = inp("ident", [P, P], BF)        # identity

    y_out = nc.dram_tensor("y", [1, TPC], dt.float32, kind="ExternalOutput")

    MM = HEADS * HOUT          # 512
    FC = MM // P               # 4 feature chunks of the GAT output
    HW = HOUT + 1              # 257: [xs_h | 1] rhs slice width

    with tile.TileContext(nc) as tc:
        with (
            tc.tile_pool(name="const", bufs=1) as cpool,
            tc.tile_pool(name="persist", bufs=1) as ppool,
            tc.tile_pool(name="dram", bufs=1, space="DRAM") as dpool,
        ):
            # ---- constants / weights to SBUF ----
            def load_const(t, shape, dtype=dt.float32):
                s = cpool.tile(shape, dtype, name=t.name, tag=t.name)
                nc.sync.dma_start(out=s[:], in_=t[:])
                return s

            def load_kc(t, k, cols, dtype=dt.float32):
                """[k*128, cols] DRAM -> [128, k, cols] SBUF."""
                s = cpool.tile([P, k, cols], dtype, name=t.name, tag=t.name)
                nc.sync.dma_start(
                    out=s[:], in_=t[:].rearrange("(k p) c -> p k c", p=P))
                return s

            negmean = load_const(negmean_in, [P, P], BF)
            posmean = load_const(posmean_in, [P, P], BF)
            w1s = load_const(w1T, [N_IN, H], BF)
            w2s = load_kc(w2T, 2, H, BF)
            gats = load_kc(gatT, 2, MM + 4, BF)
            lng = load_kc(ln_g, 2, 1)
            lnb = load_kc(ln_b, 2, 1)
            b1s = load_kc(b1, 2, 1)
            b2s = load_kc(b2, 2, 1)
            gidx_s = load_const(gidx, [P, (TILES * P) // 16], dt.int16)
            # one-hot library resident in SBUF: of-tile t = ohs[:, t*128:...]
            ohs = cpool.tile([P, TILES * P], BF, name="ohs", tag="ohs")
            ohp = cpool.tile([P, TILES * P], BF, name="ohp", tag="ohp")
            nohc = max(1, math.ceil(TILES / 16))
            for i in range(nohc):       # chunked so xb[0] isn't starved
                lo = i * (TILES * P) // nohc
                hi = (i + 1) * (TILES * P) // nohc
                nc.scalar.dma_start(out=ohs[:, lo:hi], in_=ohT_in[:, lo:hi])
                nc.scalar.dma_start(out=ohp[:, lo:hi], in_=ohP_in[:, lo:hi])
            ident = load_const(ident_in, [P, P], BF)
            skips = load_kc(skipT, 2, MM, BF)
            d1s = load_kc(d1T, 4, 4 * H, BF)
            d2s = load_kc(d2T, 8, 1, BF)
            gbia = load_kc(gat_bias, 4, 1)
            skb = load_kc(skip_b, 4, 1)
            db1s = load_kc(db1, 8, 1)
            db2s = load_const(db2, [1, 1])
            ln01 = cpool.tile([P, 1], dt.float32, name="ln01", tag="ln01")
            nc.gpsimd.memset(ln01[:], float(np.log(0.1)))

            T_tab = dpool.tile([npad, ROW], BF, name="T_tab", tag="T_tab")

            # persistent: local h2 (skip input), node-major a_dst, agg result
            h2loc = ppool.tile([P, 2, TPC], BF, name="h2loc", tag="h2loc")
            adstw = ppool.tile([P, 2 * WPC], BF, name="adstw", tag="adstw")
            aggs = ppool.tile([P, WPC, HEADS, HOUT], BF, name="aggs",
                              tag="aggs")

            # ================= Phase A: encoder -> table =================
            with (
                tc.tile_pool(name="wA", bufs=2) as wA,
                tc.tile_pool(name="asmp", bufs=2) as apool,
                tc.tile_pool(name="psA", bufs=1, space="PSUM") as psA,
                tc.tile_pool(name="psA1", bufs=1, space="PSUM") as psA1,
                tc.tile_pool(name="psA2", bufs=2, space="PSUM") as psA2,
                tc.tile_pool(name="psA3", bufs=1, space="PSUM") as psA3,
            ):
                for b in range(NB):
                    bsl = slice(b * BN, (b + 1) * BN)
                    xb = wA.tile([P, BN], BF, name="xb", tag="xb")
                    nc.sync.dma_start(out=xb[:], in_=xT[:, bsl])

                    ps2 = psA.tile([P, 2, BN], dt.float32, name="h1ps",
                                   tag="h1ps")
                    for m in range(2):
                        nc.tensor.matmul(
                            ps2[:, m, :], lhsT=w1s[:, m * P : (m + 1) * P],
                            rhs=xb[:], start=True, stop=True,
                            skip_group_check=True)
                    h1 = wA.tile([P, 2, BN], BF, name="h1", tag="h1")
                    if trivial:
                        nc.scalar.copy(h1[:], ps2[:])
                    else:
                        for m in range(2):
                            nc.scalar.activation(
                                h1[:, m, :], ps2[:, m, :], Act.Identity,
                                bias=b1s[:, m, 0:1])
                    sq = wA.tile([P, 2, BN], BF, name="sq", tag="sq")
                    nc.vector.scalar_tensor_tensor(
                        sq[:], h1[:], 1.0, h1[:],
                        op0=Alu.mult, op1=Alu.mult)

                    stats = psA1.tile([P, 2, BN], dt.float32, name="stats",
                                      tag="stats")
                    for m in range(2):
                        nc.tensor.matmul(stats[:, 0, :], lhsT=negmean[:],
                                         rhs=h1[:, m, :],
                                         start=(m == 0), stop=(m == 1),
                                         skip_group_check=True)
                        nc.tensor.matmul(stats[:, 1, :], lhsT=posmean[:],
                                         rhs=sq[:, m, :],
                                         start=(m == 0), stop=(m == 1),
                                         skip_group_check=True)
                    # stats[:,0]=-mean, stats[:,1]=E[h^2]
                    mu_bf = wA.tile([P, BN], BF, name="mu_bf", tag="mu_bf")
                    nc.vector.tensor_copy(mu_bf[:], stats[:, 0, :])
                    musq = wA.tile([P, BN], BF, name="musq", tag="musq")
                    nc.vector.scalar_tensor_tensor(
                        musq[:], mu_bf[:], 1.0, mu_bf[:],
                        op0=Alu.mult, op1=Alu.mult)
                    var = wA.tile([P, BN], dt.float32, name="var", tag="var")
                    nc.vector.scalar_tensor_tensor(
                        var[:], stats[:, 1, :], 1e-5, musq[:],
                        op0=Alu.add, op1=Alu.subtract)
                    rv = wA.tile([P, BN], dt.float32, name="rv", tag="rv")
                    nc.vector.reciprocal(rv[:], var[:])
                    rstd = wA.tile([P, BN], BF, name="rstd", tag="rstd")
                    nc.scalar.activation(rstd[:], rv[:], Act.Sqrt)

                    cen = wA.tile([P, 2, BN], BF, name="cen", tag="cen")
                    nc.vector.tensor_add(
                        cen[:], h1[:],
                        mu_bf[:].unsqueeze(1).to_broadcast([P, 2, BN]))
                    cn = wA.tile([P, 2, BN], BF, name="cn", tag="cn")
                    nc.vector.tensor_mul(
                        cn[:], cen[:],
                        rstd[:].unsqueeze(1).to_broadcast([P, 2, BN]))
                    hrelu = wA.tile([P, 2, BN], BF, name="hrelu", tag="hrelu")
                    if trivial:
                        nc.vector.tensor_scalar_max(hrelu[:], cn[:], 0.0)
                    else:
                        for m in range(2):
                            nc.scalar.activation(
                                hrelu[:, m, :], cn[:, m, :], Act.Relu,
                                bias=lnb[:, m, 0:1], scale=lng[:, m, 0:1])

                    ps2b = psA3.tile([P, 2, BN], dt.float32, name="h2ps",
                                     tag="h2ps")
                    for m in range(2):
                        for k in range(2):
                            nc.tensor.matmul(
                                ps2b[:, m, :],
                                lhsT=w2s[:, k, m * P : (m + 1) * P],
                                rhs=hrelu[:, k, :],
                                start=(k == 0), stop=(k == 1),
                                skip_group_check=True)
                    h2 = wA.tile([P, 2, BN], BF, name="h2", tag="h2")
                    if trivial:
                        nc.scalar.copy(h2[:], ps2b[:])
                    else:
                        for m in range(2):
                            nc.scalar.activation(
                                h2[:, m, :], ps2b[:, m, :], Act.Identity,
                                bias=b2s[:, m, 0:1])

                    if b * BN < TPC:  # blocks covering the local 1024 targets
                        lo = b * BN
                        nc.vector.tensor_copy(
                            h2loc[:, :, lo : lo + BN], h2[:])

                    # GAT projection, node-major: one 128-node chunk at a time
                    asm = apool.tile([P, NT, ROW], BF, name="asm", tag="asm")
                    nc.gpsimd.memset(asm[:, :, HOUT : HOUT + 1], 1.0)
                    nc.gpsimd.memset(asm[:, :, MM + 1 : MM + 2], 1.0)
                    avall = ps2b[:, 0, 0 : NT * 4].rearrange(
                        "p (t f) -> p t f", f=4)
                    for t in range(NT):
                        tsl = slice(t * P, (t + 1) * P)
                        xsps = psA2.tile([P, MM], dt.float32, name="xsps",
                                         tag="xsps")
                        for k in range(2):
                            nc.tensor.matmul(
                                xsps[:], lhsT=h2[:, k, tsl],
                                rhs=gats[:, k, 0:MM],
                                start=(k == 0), stop=(k == 1))
                        for k in range(2):
                            nc.tensor.matmul(
                                avall[:, t, :], lhsT=h2[:, k, tsl],
                                rhs=gats[:, k, MM : MM + 4],
                                start=(k == 0), stop=(k == 1),
                                skip_group_check=True)
                        # table row: [xs_h0 | 1 | xs_h1 | 1 | a_src | pad]
                        nc.scalar.copy(asm[:, t, 0:HOUT], xsps[:, 0:HOUT])
                        nc.vector.tensor_copy(asm[:, t, HOUT + 1 : MM + 1],
                                              xsps[:, HOUT:MM])
                    nc.vector.tensor_copy(asm[:, :, MM + 2 : MM + 4],
                                          avall[:, :, 0:2])
                    if b * BN < TPC:
                        nc.vector.tensor_copy(
                            adstw[:, b * 2 * NT : (b + 1) * 2 * NT]
                            .rearrange("p (t two) -> p t two", two=2),
                            avall[:, :, 2:4])
                    dst = T_tab[:].rearrange("(bb tt pp) r -> bb pp tt r",
                                             bb=NB, pp=P)[b]
                    nc.sync.dma_start(out=dst, in_=asm[:])

            # ================= Phase B: edge aggregation =================
            # Per window: half-window gathers interleaved with their
            # consumers; agg (with fused z column) accumulates in PSUM
            # across the whole window. Gathers carry row-limited in_aps so
            # early tiles can start while phase A is still writing T.
            win_t0 = []
            t0 = 0
            for w in range(WPC):
                win_t0.append(t0)
                t0 += tiles_per_win[w]
            GH = 8      # max tiles per gather call (1024 idx = 64 desc/engine)

            def _chunks(base, n):
                k = math.ceil(n / GH)
                sizes = [n // k + (1 if i < n % k else 0) for i in range(k)]
                out, b0 = [], base
                for s in sizes:
                    out.append((b0, s))
                    b0 += s
                return out

            with (
                tc.tile_pool(name="wB", bufs=3) as wB,
                tc.tile_pool(name="gpool", bufs=6) as gpool,
                tc.tile_pool(name="psB", bufs=2, space="PSUM") as psB,
            ):
                gcall = 0
                for w in range(WPC):
                    ntw = tiles_per_win[w]
                    halves = _chunks(win_t0[w], ntw)
                    agg = [psB.tile([P, HW], dt.float32, name=f"aggps{h}",
                                    tag=f"aggps{h}") for h in range(HEADS)]
                    done = 0
                    for hb, hn in halves:
                        if hn == 0:
                            continue
                        gb = gpool.tile([P, GH, ROW], BF, name="gb", tag="gb")
                        lim = rowlims[gcall] if rowlims else npad
                        nc.gpsimd.dma_gather(
                            out_ap=gb[:, :hn, :],
                            in_ap=T_tab[0:lim],
                            idxs_ap=gidx_s[:, hb * 8 : (hb + hn) * 8],
                            num_idxs=hn * P,
                            num_idxs_reg=hn * P,
                            elem_size=ROW,
                            queue_num=gcall % 4,
                        )
                        gcall += 1

                        dps = psB.tile([P, 2 * GH], dt.float32, name="dps",
                                       tag="dps")
                        for i in range(hn):
                            t = hb + i
                            nc.tensor.matmul(
                                dps[:, 2 * i : 2 * i + 2],
                                lhsT=ohs[:, t * P : (t + 1) * P],
                                rhs=adstw[:, 2 * w : 2 * w + 2],
                                start=(i == 0), stop=(i == hn - 1),
                                skip_group_check=True)
                        # e = a_src[src] + d ; leaky(0.2); exp
                        esb = wB.tile([P, GH, 2], dt.float32, name="esb",
                                      tag="esb")
                        nc.vector.tensor_add(
                            esb[:, :hn, :],
                            gb[:, :hn, MM + 2 : MM + 4],
                            dps[:, : 2 * hn].rearrange(
                                "p (t two) -> p t two", two=2))
                        lk = wB.tile([P, GH, 2], dt.float32, name="lk",
                                     tag="lk")
                        nc.vector.scalar_tensor_tensor(
                            lk[:, :hn, :], esb[:, :hn, :], 0.2,
                            esb[:, :hn, :], op0=Alu.mult, op1=Alu.max)
                        wexp = wB.tile([P, GH, 2], BF, name="wexp",
                                       tag="wexp")
                        nc.scalar.activation(wexp[:, :hn, :],
                                             lk[:, :hn, :], Act.Exp)

                        # weighted one-hots, batched per head on DVE
                        ohw = wB.tile([P, HEADS, GH, P], BF, name="ohw",
                                      tag="ohw")
                        ofv = ohp[:, hb * P : (hb + hn) * P].rearrange(
                            "p (t j) -> p t j", j=P)
                        for h in range(HEADS):
                            nc.vector.tensor_mul(
                                ohw[:, h, :hn, :], ofv,
                                wexp[:, :hn, h : h + 1]
                                .to_broadcast([P, hn, P]))

                        for i in range(hn):
                            for h in range(HEADS):
                                nc.tensor.matmul(
                                    agg[h][:],
                                    lhsT=ohw[:, h, i, :],
                                    rhs=gb[:, i, h * HW : (h + 1) * HW],
                                    start=(done == 0),
                                    stop=(done == ntw - 1),
                                    skip_group_check=True)
                            done += 1
                    # normalize: alpha = w / z  (z rode along in col 256)
                    rz = wB.tile([P, HEADS], dt.float32, name="rz", tag="rz")
                    for h in range(HEADS):
                        nc.vector.reciprocal(rz[:, h : h + 1],
                                             agg[h][:, HOUT : HOUT + 1])
                        nc.vector.tensor_scalar(
                            aggs[:, w, h, :], agg[h][:, 0:HOUT],
                            rz[:, h : h + 1], None, op0=Alu.mult)

            # ================= Phase C: epilogue =================
            with (
                tc.tile_pool(name="wC", bufs=1) as wC,
                tc.tile_pool(name="wC2", bufs=2) as wC2,
                tc.tile_pool(name="psC", bufs=2, space="PSUM") as psC,
                tc.tile_pool(name="psC2", bufs=1, space="PSUM") as psC2,
                tc.tile_pool(name="psCt", bufs=1, space="PSUM") as psCt,
            ):
                # aggs node-major [tgt, head, feat] -> convT feature-major
                convT = wC.tile([P, FC, TPC], BF, name="convT", tag="convT")
                for w in range(WPC):
                    tp = psCt.tile([P, FC, P], BF, name="tpC", tag="tpC")
                    for f in range(FC):
                        h, fo = divmod(f * P, HOUT)
                        nc.tensor.transpose(
                            tp[:, f, :], aggs[:, w, h, fo : fo + P],
                            ident[:])
                    if trivial:
                        nc.vector.tensor_copy(
                            convT[:, :, w * P : (w + 1) * P], tp[:])
                    else:
                        for f in range(FC):
                            nc.scalar.activation(
                                convT[:, f, w * P : (w + 1) * P],
                                tp[:, f, :], Act.Identity,
                                bias=gbia[:, f, 0:1])

                outT = wC.tile([P, FC, TPC], BF, name="outT", tag="outT")
                for f in range(FC):
                    sp = psC2.tile([P, 2, 512], dt.float32, name="skps",
                                   tag="skps")
                    for n in range(2):
                        for k in range(2):
                            nc.tensor.matmul(
                                sp[:, n, :],
                                lhsT=skips[:, k, f * P : (f + 1) * P],
                                rhs=h2loc[:, k, n * 512 : (n + 1) * 512],
                                start=(k == 0), stop=(k == 1),
                                skip_group_check=True)
                    cvv = convT[:, f, :].rearrange("p (n c) -> p n c", n=2)
                    t_sb = wC2.tile([P, 2, 512], BF, name="t_sb",
                                    tag="t_sb")
                    if trivial:
                        nc.vector.tensor_add(t_sb[:], sp[:], cvv)
                    else:
                        nc.vector.scalar_tensor_tensor(
                            t_sb[:], sp[:], skb[:, f, 0:1], cvv,
                            op0=Alu.add, op1=Alu.add)
                    mn = wC2.tile([P, 2, 512], BF, name="mn", tag="mn")
                    nc.vector.tensor_scalar_min(mn[:], t_sb[:], 0.0)
                    ez = wC2.tile([P, 2, 512], dt.float32, name="ez",
                                  tag="ez")
                    nc.scalar.activation(ez[:], mn[:], Act.Exp,
                                         bias=ln01[:, 0:1])
                    rl = wC2.tile([P, 2, 512], dt.float32, name="rl",
                                  tag="rl")
                    nc.scalar.activation(rl[:], t_sb[:], Act.Relu)
                    nc.vector.scalar_tensor_tensor(
                        outT[:, f, :].rearrange("p (n c) -> p n c", n=2),
                        ez[:], -0.1, rl[:], op0=Alu.add, op1=Alu.add)

                dsb = wC.tile([P, 8, TPC], BF, name="dsb", tag="dsb")
                for m in range(8):
                    ps = psC.tile([P, 2, 512], dt.float32, name="decps",
                                  tag="decps")
                    for n in range(2):
                        for k in range(FC):
                            nc.tensor.matmul(
                                ps[:, n, :],
                                lhsT=d1s[:, k, m * P : (m + 1) * P],
                                rhs=outT[:, k, n * 512 : (n + 1) * 512],
                                start=(k == 0), stop=(k == FC - 1),
                                skip_group_check=True)
                    tmp = wC2.tile([P, 2, 512], BF, name="dtmp",
                                   tag="dtmp")
                    if trivial:
                        nc.scalar.copy(tmp[:], ps[:])
                    else:
                        nc.scalar.activation(tmp[:], ps[:], Act.Identity,
                                             bias=db1s[:, m, 0:1])
                    nc.vector.scalar_tensor_tensor(
                        dsb[:, m, :].rearrange("p (n c) -> p n c", n=2),
                        tmp[:], 0.1, tmp[:], op0=Alu.mult, op1=Alu.max)

                ysb = wC.tile([1, TPC], dt.float32, name="ysb", tag="ysb")
                for n in range(2):
                    nsl = slice(n * 512, (n + 1) * 512)
                    yp = psC.tile([1, 512], dt.float32, name="yps",
                                  tag="yps", bufs=1)
                    for m in range(8):
                        nc.tensor.matmul(
                            yp[:], lhsT=d2s[:, m, 0:1],
                            rhs=dsb[:, m, nsl],
                            start=(m == 0), stop=(m == 7))
                    nc.scalar.activation(ysb[:, nsl], yp[:], Act.Identity,
                                         bias=db2s[0:1, 0:1])
                nc.sync.dma_start(out=y_out[:], in_=ysb[:])

    nc.compile()
    return nc


def _build_program_fast(tiles_per_win, nb, rowlims):
    """Trivial-parameter (zero biases, unit LN gain) builder.

    Phase A runs node-major: LayerNorm stats via DVE bn_stats, the
    normalize+ReLU fused into one per-chunk scalar-engine activation
    (per-partition scale/bias) writing straight into the table row, and
    enc_w2 folded on the host into the GAT/skip projection weights.
    The table stores 256-dim hrelu (768 B rows) — the GAT projection
    commutes with the attention-weighted sum, so it runs after phase B
    on the 1024 aggregated vectors instead of on all 10k nodes.
    """
    import concourse.bacc as bacc
    import concourse.mybir as mybir
    import concourse.tile as tile

    dt = mybir.dt
    Alu = mybir.AluOpType
    Act = mybir.ActivationFunctionType
    BF = dt.bfloat16

    TILES = sum(tiles_per_win)
    NB = nb                     # node blocks in phase A (compacted)
    BN = 512                    # nodes per block
    NT = BN // P                # 4 node chunks of 128 per block
    npad = NB * BN              # compacted node count

    nc = bacc.Bacc("TRN2", target_bir_lowering=False, num_swdge_queues=4)

    def inp(name, shape, dtype=dt.float32):
        return nc.dram_tensor(name, shape, dtype, kind="ExternalInput")

    xT = inp("xT", [P, npad], BF)
    w1T = inp("w1T", [N_IN, H], BF)
    gatT = inp("gatT", [H, HEADS * HOUT + 4], BF)  # (gat_w@w2).T + att cols
    skipT = inp("skipT", [H, HEADS * HOUT], BF)    # (skip_w@w2).T
    d1T = inp("d1T", [HEADS * HOUT, 4 * H], BF)
    d2T = inp("d2T", [4 * H, 1], BF)
    gidx = inp("gidx", [P, (TILES * P) // 16], dt.int16)
    ohT_in = inp("ohT", [P, TILES * P], BF)    # [j, t*128+p] one-hots
    ohP_in = inp("ohP", [P, TILES * P], BF)    # [p, t*128+j] one-hots
    ident_in = inp("ident", [P, P], BF)        # identity

    y_out = nc.dram_tensor("y", [1, TPC], dt.float32, kind="ExternalOutput")

    MM = HEADS * HOUT          # 512
    FC = MM // P               # 4 feature chunks of the GAT output
    HW = HOUT + 1              # 257: [hrelu | 1] agg rhs slice width
    ASRC = HW                  # a_src pair at row cols 257:259

    with tile.TileContext(nc) as tc:
        with (
            tc.tile_pool(name="const", bufs=1) as cpool,
            tc.tile_pool(name="persist", bufs=1) as ppool,
            tc.tile_pool(name="dram", bufs=1, space="DRAM") as dpool,
        ):
            # ---- constants / weights to SBUF ----
            def load_const(t, shape, dtype=dt.float32):
                s = cpool.tile(shape, dtype, name=t.name, tag=t.name)
                nc.sync.dma_start(out=s[:], in_=t[:])
                return s

            def load_kc(t, k, cols, dtype=dt.float32):
                """[k*128, cols] DRAM -> [128, k, cols] SBUF."""
                s = cpool.tile([P, k, cols], dtype, name=t.name, tag=t.name)
                nc.sync.dma_start(
                    out=s[:], in_=t[:].rearrange("(k p) c -> p k c", p=P))
                return s

            w1s = load_const(w1T, [N_IN, H], BF)
            gats = load_kc(gatT, 2, MM + 4, BF)
            gidx_s = cpool.tile([P, (TILES * P) // 16], dt.int16,
                                name="gidx", tag="gidx")
            # one-hot library resident in SBUF: of-tile t = ohs[:, t*128:...]
            ohs = cpool.tile([P, TILES * P], BF, name="ohs", tag="ohs")
            ohp = cpool.tile([P, TILES * P], BF, name="ohp", tag="ohp")
            ident = load_const(ident_in, [P, P], BF)

            def decl_kc(t, k, cols, dtype=dt.float32):
                return cpool.tile([P, k, cols], dtype, name=t.name,
                                  tag=t.name)

            # declared now, loaded inside the phase A loop (off the
            # startup critical path)
            skips = decl_kc(skipT, 2, MM, BF)
            d1s = decl_kc(d1T, 4, 4 * H, BF)
            d2s = decl_kc(d2T, 8, 1, BF)
            ln01 = cpool.tile([P, 1], dt.float32, name="ln01", tag="ln01")
            nc.gpsimd.memset(ln01[:], float(np.log(0.1)))

            T_tab = dpool.tile([npad, ROWF], BF, name="T_tab", tag="T_tab")

            # persistent: node-major a_dst, skip output, agg result
            adstw = ppool.tile([P, 2 * WPC], BF, name="adstw", tag="adstw")
            skloc = ppool.tile([P, WPC, MM], BF, name="skloc", tag="skloc")
            aggs = ppool.tile([P, WPC, HEADS, HOUT], BF, name="aggs",
                              tag="aggs")

            # ================= Phase A: encoder -> table =================
            with (
                tc.tile_pool(name="wA", bufs=2) as wA,
                tc.tile_pool(name="asmp", bufs=2) as apool,
                tc.tile_pool(name="psH", bufs=4, space="PSUM") as psH,
                tc.tile_pool(name="psT", bufs=2, space="PSUM") as psT,
                tc.tile_pool(name="psSk", bufs=1, space="PSUM") as psSk,
                tc.tile_pool(name="psAv", bufs=1, space="PSUM") as psAv,
            ):
                nohc = max(1, min(NB - 2, 10))
                for b in range(NB):
                    bsl = slice(b * BN, (b + 1) * BN)
                    local = b * BN < TPC
                    xb = wA.tile([P, BN], BF, name="xb", tag="xb")
                    nc.sync.dma_start(out=xb[:], in_=xT[:, bsl])
                    if b < nohc:
                        # stream the one-hot library during phase A,
                        # splitting queue residency across SP and ACT
                        lo = b * (TILES * P) // nohc
                        hi = (b + 1) * (TILES * P) // nohc
                        qa = nc.sync if b % 2 == 0 else nc.scalar
                        qb = nc.scalar if b % 2 == 0 else nc.sync
                        qa.dma_start(out=ohs[:, lo:hi],
                                     in_=ohT_in[:, lo:hi])
                        qb.dma_start(out=ohp[:, lo:hi],
                                     in_=ohP_in[:, lo:hi])

                    if b == 0:
                        nc.scalar.dma_start(
                            out=skips[:],
                            in_=skipT[:].rearrange("(k p) c -> p k c", p=P))
                        nc.scalar.dma_start(
                            out=gidx_s[:], in_=gidx[:])
                    elif b == 1:
                        nc.scalar.dma_start(
                            out=d1s[:],
                            in_=d1T[:].rearrange("(k p) c -> p k c", p=P))
                        nc.scalar.dma_start(
                            out=d2s[:],
                            in_=d2T[:].rearrange("(k p) c -> p k c", p=P))
                    hreluT = wA.tile([P, NT, 2, P], BF, name="hreluT",
                                     tag="hreluT")
                    st6 = wA.tile([P, NT, 6], dt.float32, name="st6",
                                  tag="st6")
                    mv = wA.tile([P, NT, 2], dt.float32, name="mv", tag="mv")
                    varp = wA.tile([P, NT], dt.float32, name="varp",
                                   tag="varp")
                    rv = wA.tile([P, NT], dt.float32, name="rv", tag="rv")
                    rstd = wA.tile([P, NT], dt.float32, name="rstd",
                                   tag="rstd")
                    nmr = wA.tile([P, NT], dt.float32, name="nmr", tag="nmr")
                    av4 = psAv.tile([P, NT, 4], dt.float32, name="av4",
                                    tag="av4")
                    asm = apool.tile([P, NT, ROWF], BF, name="asm", tag="asm")
                    nc.vector.memset(asm[:, :, HOUT : HOUT + 1], 1.0)

                    h1s = []
                    for g in range(2):          # halves of the block
                        h1ps = psH.tile([P, 2, H], dt.float32, name="h1ps",
                                        tag="h1ps")
                        h1s.append(h1ps)
                        for tt in range(2):
                            t = 2 * g + tt
                            nc.tensor.matmul(
                                h1ps[:, tt, :],
                                lhsT=xb[:, t * P : (t + 1) * P],
                                rhs=w1s[:], start=True, stop=True,
                                skip_group_check=True)
                        for tt in range(2):
                            t = 2 * g + tt
                            nc.vector.bn_stats(st6[:, t, :],
                                               h1ps[:, tt, :])
                            nc.vector.bn_aggr(mv[:, t, :], st6[:, t, :])
                    # batched LN smalls for all 4 chunks
                    nc.vector.tensor_scalar(
                        varp[:], mv[:, :, 1], 1e-5, None, op0=Alu.add)
                    nc.vector.reciprocal(rv[:], varp[:])
                    nc.scalar.activation(rstd[:], rv[:], Act.Sqrt)
                    nc.vector.scalar_tensor_tensor(
                        nmr[:], mv[:, :, 0], -1.0, rstd[:],
                        op0=Alu.mult, op1=Alu.mult)
                    tp = psT.tile([P, NT, 2, P], BF, name="tpA", tag="tpA")
                    for t in range(NT):
                        # normalize+relu straight into the table row
                        nc.scalar.activation(
                            asm[:, t, 0:H], h1s[t // 2][:, t % 2, :],
                            Act.Relu, bias=nmr[:, t : t + 1],
                            scale=rstd[:, t : t + 1])
                        for m in range(2):
                            nc.tensor.transpose(
                                tp[:, t, m, :],
                                asm[:, t, m * P : (m + 1) * P],
                                ident[:])
                    nc.vector.tensor_copy(hreluT[:], tp[:])

                    for t in range(NT):
                        for k in range(2):
                            nc.tensor.matmul(
                                av4[:, t, :], lhsT=hreluT[:, t, k, :],
                                rhs=gats[:, k, MM : MM + 4],
                                start=(k == 0), stop=(k == 1),
                                skip_group_check=True)
                        if local:
                            w = b * NT + t
                            skps = psSk.tile([P, MM], dt.float32,
                                             name="skps", tag="skps")
                            for k in range(2):
                                nc.tensor.matmul(
                                    skps[:], lhsT=hreluT[:, t, k, :],
                                    rhs=skips[:, k, :],
                                    start=(k == 0), stop=(k == 1))
                            nc.vector.tensor_copy(skloc[:, w, :], skps[:])
                    nc.vector.tensor_copy(asm[:, :, ASRC : ASRC + 2],
                                          av4[:, :, 0:2])
                    if local:
                        nc.vector.tensor_copy(
                            adstw[:, b * 2 * NT : (b + 1) * 2 * NT]
                            .rearrange("p (t two) -> p t two", two=2),
                            av4[:, :, 2:4])
                    # row b*512 + p*4 + t  (contiguous 4-row run / partition)
                    dst = T_tab[b * BN : (b + 1) * BN].rearrange(
                        "(pp tt) r -> pp tt r", tt=NT)
                    nc.sync.dma_start(out=dst, in_=asm[:])

            # ================= Phase B: edge aggregation =================
            win_t0 = []
            t0 = 0
            for w in range(WPC):
                win_t0.append(t0)
                t0 += tiles_per_win[w]
            GH = 8      # max tiles per gather call (1024 idx = 64 desc/engine)

            def _chunks(base, n):
                k = math.ceil(n / GH)
                sizes = [n // k + (1 if i < n % k else 0) for i in range(k)]
                out, b0 = [], base
                for s in sizes:
                    out.append((b0, s))
                    b0 += s
                return out

            with (
                tc.tile_pool(name="wB", bufs=3) as wB,
                tc.tile_pool(name="gpool", bufs=6) as gpool,
                tc.tile_pool(name="psB", bufs=2, space="PSUM") as psB,
            ):
                gcall = 0
                for w in range(WPC):
                    ntw = tiles_per_win[w]
                    halves = _chunks(win_t0[w], ntw)
                    agg = [psB.tile([P, HW], dt.float32, name=f"aggps{h}",
                                    tag=f"aggps{h}") for h in range(HEADS)]
                    done = 0
                    for hb, hn in halves:
                        if hn == 0:
                            continue
                        gb = gpool.tile([P, GH, ROWF], BF, name="gb", tag="gb")
                        lim = rowlims[gcall] if rowlims else npad
                        nc.gpsimd.dma_gather(
                            out_ap=gb[:, :hn, :],
                            in_ap=T_tab[0:lim],
                            idxs_ap=gidx_s[:, hb * 8 : (hb + hn) * 8],
                            num_idxs=hn * P,
                            num_idxs_reg=hn * P,
                            elem_size=ROWF,
                            queue_num=gcall % 4,
                        )
                        gcall += 1

                        dps = psB.tile([P, 2 * GH], dt.float32, name="dps",
                                       tag="dps")
                        for i in range(hn):
                            t = hb + i
                            nc.tensor.matmul(
                                dps[:, 2 * i : 2 * i + 2],
                                lhsT=ohs[:, t * P : (t + 1) * P],
                                rhs=adstw[:, 2 * w : 2 * w + 2],
                                start=(i == 0), stop=(i == hn - 1),
                                skip_group_check=True)
                        # e = a_src[src] + d ; leaky(0.2); exp
                        esb = wB.tile([P, GH, 2], dt.float32, name="esb",
                                      tag="esb")
                        nc.vector.tensor_add(
                            esb[:, :hn, :],
                            gb[:, :hn, ASRC : ASRC + 2],
                            dps[:, : 2 * hn].rearrange(
                                "p (t two) -> p t two", two=2))
                        lk = wB.tile([P, GH, 2], dt.float32, name="lk",
                                     tag="lk")
                        nc.vector.scalar_tensor_tensor(
                            lk[:, :hn, :], esb[:, :hn, :], 0.2,
                            esb[:, :hn, :], op0=Alu.mult, op1=Alu.max)
                        wexp = wB.tile([P, GH, 2], dt.float32,
                                       name="wexp", tag="wexp")
                        nc.scalar.activation(wexp[:, :hn, :],
                                             lk[:, :hn, :], Act.Exp)

                        # weighted one-hots: per-(tile,head) tensor_scalar
                        # (per-partition scalar ptr -> DVE 4x mode)
                        ohw = wB.tile([P, HEADS, GH, P], BF, name="ohw",
                                      tag="ohw")
                        for i in range(hn):
                            t = hb + i
                            for h in range(HEADS):
                                if (i * HEADS + h) % 4 == 3:
                                    nc.scalar.activation(
                                        ohw[:, h, i, :],
                                        ohp[:, t * P : (t + 1) * P],
                                        Act.Copy,
                                        scale=wexp[:, i, h : h + 1])
                                else:
                                    nc.vector.tensor_scalar(
                                        ohw[:, h, i, :],
                                        ohp[:, t * P : (t + 1) * P],
                                        wexp[:, i, h : h + 1], None,
                                        op0=Alu.mult)

                        for i in range(hn):
                            for h in range(HEADS):
                                nc.tensor.matmul(
                                    agg[h][:],
                                    lhsT=ohw[:, h, i, :],
                                    rhs=gb[:, i, 0:HW],
                                    start=(done == 0),
                                    stop=(done == ntw - 1),
                                    skip_group_check=True)
                            done += 1
                    # normalize: alpha = w / z  (z rode along in col 256)
                    rz = wB.tile([P, HEADS], dt.float32, name="rz", tag="rz")
                    for h in range(HEADS):
                        nc.vector.reciprocal(rz[:, h : h + 1],
                                             agg[h][:, HOUT : HOUT + 1])
                        nc.vector.tensor_scalar(
                            aggs[:, w, h, :], agg[h][:, 0:HOUT],
                            rz[:, h : h + 1], None, op0=Alu.mult)

            # ================= Phase C: epilogue =================
            # agg holds weighted-summed hrelu vectors; project them through
            # the folded GAT weights here (projection commutes with the sum),
            # add skip, ELU, transpose, and run the decoder.
            with (
                tc.tile_pool(name="wC", bufs=1) as wC,
                tc.tile_pool(name="wC2", bufs=3) as wC2,
                tc.tile_pool(name="psC", bufs=2, space="PSUM") as psC,
                tc.tile_pool(name="psCv", bufs=1, space="PSUM") as psCv,
                tc.tile_pool(name="psCh", bufs=2, space="PSUM") as psCh,
                tc.tile_pool(name="psCt", bufs=2, space="PSUM") as psCt,
            ):
                outT = wC.tile([P, FC, TPC], BF, name="outT", tag="outT")
                dsb = wC.tile([P, 8, TPC], BF, name="dsb", tag="dsb")
                ysb = wC.tile([1, TPC], dt.float32, name="ysb", tag="ysb")
                for w in range(WPC):
                    wsl = slice(w * P, (w + 1) * P)
                    # transpose the aggregated hrelu (both heads)
                    tph = psCh.tile([P, HEADS, 2, P], BF, name="tph",
                                    tag="tph")
                    for h in range(HEADS):
                        for m in range(2):
                            nc.tensor.transpose(
                                tph[:, h, m, :],
                                aggs[:, w, h, m * P : (m + 1) * P],
                                ident[:])
                    aggT = wC2.tile([P, HEADS, 2, P], BF, name="aggT",
                                    tag="aggT")
                    nc.scalar.copy(aggT[:], tph[:])
                    # conv = Wc @ agg-hrelu  (per head), both into one bank
                    cv = psCv.tile([P, MM], dt.float32, name="cv", tag="cv")
                    for h in range(HEADS):
                        for m in range(2):
                            nc.tensor.matmul(
                                cv[:, h * HOUT : (h + 1) * HOUT],
                                lhsT=aggT[:, h, m, :],
                                rhs=gats[:, m, h * HOUT : (h + 1) * HOUT],
                                start=(m == 0), stop=(m == 1),
                                skip_group_check=True)
                    t_sb = wC2.tile([P, MM], BF, name="t_sb", tag="t_sb")
                    nc.vector.tensor_add(t_sb[:], cv[:], skloc[:, w, :])
                    mn = wC2.tile([P, MM], BF, name="mn", tag="mn")
                    nc.vector.tensor_scalar_min(mn[:], t_sb[:], 0.0)
                    ez = wC2.tile([P, MM], dt.float32, name="ez", tag="ez")
                    nc.scalar.activation(ez[:], mn[:], Act.Exp,
                                         bias=ln01[:, 0:1])
                    rl = wC2.tile([P, MM], BF, name="rl", tag="rl")
                    nc.vector.tensor_scalar_max(rl[:], t_sb[:], 0.0)
                    outN = wC2.tile([P, MM], BF, name="outN", tag="outN")
                    nc.vector.scalar_tensor_tensor(
                        outN[:], ez[:], -0.1, rl[:],
                        op0=Alu.add, op1=Alu.add)
                    tp = psCt.tile([P, FC, P], BF, name="tpC", tag="tpC")
                    for f in range(FC):
                        nc.tensor.transpose(
                            tp[:, f, :], outN[:, f * P : (f + 1) * P],
                            ident[:])
                    nc.scalar.copy(outT[:, :, wsl], tp[:])

                    if w % 4 == 3:
                        # decoder half over the 4 windows just finished
                        n = w // 4
                        nsl = slice(n * 512, (n + 1) * 512)
                        for m in range(8):
                            ps = psC.tile([P, 512], dt.float32,
                                          name="decps", tag="decps")
                            for k in range(FC):
                                nc.tensor.matmul(
                                    ps[:],
                                    lhsT=d1s[:, k, m * P : (m + 1) * P],
                                    rhs=outT[:, k, nsl],
                                    start=(k == 0), stop=(k == FC - 1),
                                    skip_group_check=True)
                            tmp = wC2.tile([P, 512], BF, name="dtmp",
                                           tag="dtmp")
                            nc.scalar.copy(tmp[:], ps[:])
                            nc.vector.scalar_tensor_tensor(
                                dsb[:, m, nsl], tmp[:], 0.1, tmp[:],
                                op0=Alu.mult, op1=Alu.max)
                        yp = psC.tile([1, 512], dt.float32, name="yps",
                                      tag="yps", bufs=1)
                        for m in range(8):
                            nc.tensor.matmul(
                                yp[:], lhsT=d2s[:, m, 0:1],
                                rhs=dsb[:, m, nsl],
                                start=(m == 0), stop=(m == 7))
                        nc.scalar.copy(ysb[:, nsl], yp[:])
                nc.sync.dma_start(out=y_out[:], in_=ysb[:])

    nc.compile()
    return nc


def _build_program(tiles_per_win, nb, trivial, rowlims):
    if trivial:
        return _build_program_fast(tiles_per_win, nb, rowlims)
    return _build_program_legacy(tiles_per_win, nb, trivial, rowlims)


# ----------------------------------------------------------------------------
# Driver
# ----------------------------------------------------------------------------

def _consts():
    negmean = np.full((P, P), -1.0 / H, dtype=BF16)
    posmean = np.full((P, P), 1.0 / H, dtype=BF16)
    ident = np.eye(P, dtype=BF16)
    return negmean, posmean, ident


def _host_in_maps(inputs, buckets, tiles_per_win):
    x = np.asarray(inputs["x"], dtype=F32)
    enc_w1, enc_b1 = inputs["enc_w1"], inputs["enc_b1"]
    ln_g, ln_b = inputs["ln_g"], inputs["ln_b"]
    enc_w2, enc_b2 = inputs["enc_w2"], inputs["enc_b2"]
    gat_w, att_src, att_dst = inputs["gat_w"], inputs["att_src"], inputs["att_dst"]
    gat_bias, skip_w, skip_b = inputs["gat_bias"], inputs["skip_w"], inputs["skip_b"]
    dec_w1, dec_b1 = inputs["dec_w1"], inputs["dec_b1"]
    dec_w2, dec_b2 = inputs["dec_w2"], inputs["dec_b2"]

    trivial = bool(
        all(np.all(np.asarray(v, F32) == 0.0)
            for v in (enc_b1, enc_b2, ln_b, gat_bias, skip_b, dec_b1,
                      dec_b2))
        and np.all(np.asarray(ln_g, F32) == 1.0))

    # a_src[n,h] = att_src[h] . xs[n,h,:] = (gat_w[h-block].T @ att_src[h]) . h2
    # -> compose the attention vectors into h2-space columns on the host.
    gw = np.asarray(gat_w, F32)
    w2 = np.asarray(enc_w2, F32)
    sw = np.asarray(skip_w, F32)
    asr = np.asarray(att_src, F32).reshape(HEADS, HOUT)
    ads = np.asarray(att_dst, F32).reshape(HEADS, HOUT)

    negmean, posmean, ident = _consts()
    col = lambda v: np.ascontiguousarray(np.asarray(v, F32).reshape(-1, 1))
    bf = lambda v: np.ascontiguousarray(np.asarray(v, F32)).astype(BF16)
    common = {
        "w1T": bf(np.asarray(enc_w1, F32).T),
        "d1T": bf(np.asarray(dec_w1, F32).T),
        "d2T": bf(np.asarray(dec_w2, F32).T),
        "ident": ident,
    }
    if trivial:
        # fold enc_w2 into the GAT / skip projections (hrelu-space)
        gwc = gw @ w2                                 # [512, 256]
        att = np.zeros((H, 4), dtype=F32)
        for h in range(HEADS):
            blk = gwc[h * HOUT : (h + 1) * HOUT, :]   # [HOUT, H]
            att[:, h] = blk.T @ asr[h]
            att[:, 2 + h] = blk.T @ ads[h]
        common["gatT"] = bf(np.concatenate(
            [np.ascontiguousarray(gwc.T), att], axis=1))
        common["skipT"] = bf(np.ascontiguousarray((sw @ w2).T))
    else:
        att = np.zeros((H, 4), dtype=F32)
        for h in range(HEADS):
            blk = gw[h * HOUT : (h + 1) * HOUT, :]    # [HOUT, H]
            att[:, h] = blk.T @ asr[h]
            att[:, 2 + h] = blk.T @ ads[h]
        common.update({
            "ln_g": col(ln_g), "ln_b": col(ln_b),
            "b1": col(enc_b1), "b2": col(enc_b2),
            "w2T": bf(w2.T),
            "gatT": bf(np.concatenate(
                [np.ascontiguousarray(gw.T), att], axis=1)),
            "skipT": bf(sw.T),
            "gat_bias": col(gat_bias), "skip_b": col(skip_b),
            "db1": col(dec_b1), "db2": col(dec_b2),
            "negmean": negmean, "posmean": posmean,
        })

    orders = [_per_core_order(buckets, c) for c in range(NCORES)]
    nb = int(math.ceil(max(o.size for o in orders) / 512))
    npad = nb * 512

    in_maps = []
    tmaxes = []
    for c in range(NCORES):
        order = orders[c]
        wrapped, ohJ, ohP, tmax = _per_core_arrays(
            buckets, tiles_per_win, c, order, pmajor=trivial)
        tmaxes.append(tmax)
        m = dict(common)
        xp = np.zeros((npad, N_IN), dtype=F32)
        xp[:order.size] = x[order]
        m["xT"] = np.ascontiguousarray(xp.T).astype(BF16)
        m["gidx"] = wrapped
        m["ohT"] = ohJ
        m["ohP"] = ohP
        in_maps.append(m)

    # per-gather-call row limits (same call order as the builder emits)
    tmax_all = np.max(np.stack(tmaxes), axis=0)      # per tile, over cores
    GH = 8
    rowlims = []
    t0 = 0
    for w in range(WPC):
        ntw = tiles_per_win[w]
        k = math.ceil(ntw / GH)
        sizes = [ntw // k + (1 if i < ntw % k else 0) for i in range(k)]
        b0 = t0
        for sz in sizes:
            hi = int(tmax_all[b0 : b0 + sz].max()) + 1
            rowlims.append(min(int(math.ceil(hi / 512.0)) * 512, npad))
            b0 += sz
        t0 += ntw
    return in_maps, nb, trivial, rowlims


def kernel(x, edge_index, batch_size, enc_w1, enc_b1, ln_g, ln_b, enc_w2,
           enc_b2, gat_w, att_src, att_dst, gat_bias, skip_w, skip_b,
           dec_w1, dec_b1, dec_w2, dec_b2, _trace=False):
    edge_index = np.asarray(edge_index)
    buckets, tiles_per_win = _prepare_edges(edge_index)

    inputs = dict(x=x, enc_w1=enc_w1, enc_b1=enc_b1, ln_g=ln_g, ln_b=ln_b,
                  enc_w2=enc_w2, enc_b2=enc_b2, gat_w=gat_w, att_src=att_src,
                  att_dst=att_dst, gat_bias=gat_bias, skip_w=skip_w,
                  skip_b=skip_b, dec_w1=dec_w1, dec_b1=dec_b1, dec_w2=dec_w2,
                  dec_b2=dec_b2)
    in_maps, nb, trivial, rowlims = _host_in_maps(
        inputs, buckets, tiles_per_win)

    key = (tuple(tiles_per_win), nb, trivial, tuple(rowlims))
    if key not in _cache:
        _cache[key] = _build_program(tiles_per_win, nb, trivial, rowlims)
    nc = _cache[key]

    from concourse.bass_utils import run_bass_kernel_spmd
    res = run_bass_kernel_spmd(
        nc, in_maps, core_ids=list(range(NCORES)), trace=_trace)

    y = np.concatenate([res.results[c]["y"][0] for c in range(NCORES)])
    out = y.reshape(BATCH, 1).astype(F32)
    if _trace:
        return out, res
    return out

